# revision 18
# baseline (speedup 1.0000x reference)
"""Block sliding-window attention on 8 TRN2 NeuronCores.

Sharding: sequence-parallel. 8 shards = (batch b in {0,1}) x (quarter s in
0..3); each core owns 2048 consecutive tokens of one batch plus a 256-token
K/V halo from the previous quarter (zeros + -inf gate for the first quarter).
No collectives: each core computes its tokens' full output rows.

Per-core pipeline (all matmuls in float32r: full PE rate, ~1e-4 rounding):
  P1: QT/KT = W^T @ hiddenT (head-transposed layout, raw), V = hidden @ Wv
      (natural layout), all staged through DRAM scratch.
  P2: per 256-token chunk: RoPE on Q/K (rot-half via partition-offset DMA
      reload + pre-signed sin), then per head: S^T = K Q^T per 128-key block,
      exp on ACT (scale=1/sqrt(128), -1e30 bias gates the no-previous case),
      0/1 triangular mask multiply on DVE (also retypes to f32r), denominator
      via all-ones matmul (broadcasts across partitions), O^T = V^T P^T,
      normalize with DVE reciprocal.
  P3: out = sum_h O_h @ Wo_h, accumulated over all 16 head blocks in PSUM.
"""
import sys

try:
    import concourse  # noqa: F401
except ImportError:
    sys.path.insert(0, '/opt/trn_rl_repo')

import ml_dtypes
import numpy as np

import concourse.bacc as bacc
import concourse.mybir as mybir
import concourse.tile as tile
from concourse.bass_utils import run_bass_kernel_spmd

f32 = mybir.dt.float32
f32r = mybir.dt.float32r
AF = mybir.ActivationFunctionType
bf16 = mybir.dt.bfloat16

DIMS = 2048
HEADS = 16
HD = 128           # head dim
WIN = 256          # window / chunk
B, S = 2, 8192
NSH = 4            # seq shards per batch
THETA = 10000.0
ISQ = float(1.0 / np.sqrt(HD))
IB = DIMS // 128   # 16 input-dim blocks


def build(nc, T, phases=(1, 2, 3)):
    """Emit the per-core program. T = local tokens (multiple of 512)."""
    TH = T + WIN                      # with halo
    NC_ = T // WIN                    # chunks
    HT = nc.dram_tensor("HT", [DIMS, TH], f32r, kind="ExternalInput")
    WQ = nc.dram_tensor("WQ", [DIMS, DIMS], f32r, kind="ExternalInput")
    WK = nc.dram_tensor("WK", [DIMS, DIMS], f32r, kind="ExternalInput")
    WV = nc.dram_tensor("WV", [DIMS, DIMS], f32r, kind="ExternalInput")
    WO = nc.dram_tensor("WO", [DIMS, DIMS], f32r, kind="ExternalInput")
    COS = nc.dram_tensor("COS", [HD, TH], f32, kind="ExternalInput")
    SINS = nc.dram_tensor("SINS", [HD, TH], f32, kind="ExternalInput")
    TRI23 = nc.dram_tensor("TRI23", [128, 2 * WIN], f32, kind="ExternalInput")
    PGATE = nc.dram_tensor("PGATE", [128, 1], f32, kind="ExternalInput")
    ONESM = nc.dram_tensor("ONESM", [128, 128], bf16, kind="ExternalInput")
    OUT = nc.dram_tensor("OUT", [T, DIMS], f32, kind="ExternalOutput")

    QTS = nc.dram_tensor("QTS", [HEADS, HD, T], f32)    # raw (pre-RoPE) Q^T
    KTS = nc.dram_tensor("KTS", [HEADS, HD, TH], f32)   # raw K^T (with halo)
    VS = nc.dram_tensor("VS", [TH, DIMS], bf16)         # V natural
    OTS = nc.dram_tensor("OTS", [HEADS, HD, T], f32r)   # normalized O^T

    def tok_tiles(n):
        out, a = [], 0
        while a < n:
            w = min(512, n - a)
            out.append((a, w))
            a += w
        return out

    with tile.TileContext(nc) as tc:
        with tc.tile_pool(name="cst", bufs=1) as cst:
            tri23 = cst.tile([128, 2 * WIN], f32)
            pgate = cst.tile([128, 1], f32)
            onesm = cst.tile([128, 128], bf16)
            nc.sync.dma_start(tri23[:], TRI23[:])
            nc.sync.dma_start(pgate[:], PGATE[:])
            nc.sync.dma_start(onesm[:], ONESM[:])

            # ---------------- P1: projections ----------------
            if 1 in phases:
              with tc.tile_pool(name="p1", bufs=1) as p1, \
                 tc.tile_pool(name="wp", bufs=8) as wp, \
                 tc.tile_pool(name="st", bufs=6) as st, \
                 tc.tile_pool(name="pp", bufs=8, space="PSUM") as pp:
                ht = p1.tile([128, IB, TH], f32r)
                nc.sync.dma_start(ht[:], HT.rearrange("(ib p) t -> p ib t", p=128))

                # QT / KT: lhsT = W block [128in, 128out], rhs = hT
                for W_, DST, t0, tlen in ((WQ, QTS, WIN, T), (WK, KTS, 0, TH)):
                    for ob in range(HEADS):
                        tts = tok_tiles(tlen)
                        psums = [pp.tile([128, 512], f32, name="pp") for _ in tts]
                        for ib in range(IB):
                            wt = wp.tile([128, 128], f32r, name="w")
                            nc.sync.dma_start(
                                wt[:], W_[ib * 128:(ib + 1) * 128,
                                          ob * 128:(ob + 1) * 128])
                            for ti, (a, w) in enumerate(tts):
                                nc.tensor.matmul(
                                    psums[ti][:, :w], wt[:],
                                    ht[:, ib, t0 + a:t0 + a + w],
                                    start=(ib == 0), stop=(ib == IB - 1))
                        for ti, (a, w) in enumerate(tts):
                            so = st.tile([128, 512], f32, name="st")
                            nc.scalar.copy(so[:, :w], psums[ti][:, :w])
                            nc.sync.dma_start(DST[ob][:, a:a + w], so[:, :w])

                # V natural: lhsT = hT block [128in, 128tok], rhs = Wv rows
                NTB = TH // 128
                for tb0 in range(0, NTB, 4):
                    tbs = list(range(tb0, min(tb0 + 4, NTB)))
                    for og in range(4):
                        psums = {}
                        for ib in range(IB):
                            wt = wp.tile([128, 512], f32r, name="wv")
                            nc.sync.dma_start(
                                wt[:], WV[ib * 128:(ib + 1) * 128,
                                          og * 512:(og + 1) * 512])
                            for tb in tbs:
                                if ib == 0:
                                    psums[tb] = pp.tile([128, 512], f32, name="pp")
                                nc.tensor.matmul(
                                    psums[tb][:],
                                    ht[:, ib, tb * 128:(tb + 1) * 128], wt[:],
                                    start=(ib == 0), stop=(ib == IB - 1))
                        for tb in tbs:
                            so = st.tile([128, 512], bf16, name="stv")
                            nc.vector.tensor_copy(so[:], psums[tb][:])
                            nc.sync.dma_start(
                                VS[tb * 128:(tb + 1) * 128,
                                   og * 512:(og + 1) * 512], so[:])

            # ---------------- P2: attention ----------------
            if 2 in phases:
              with tc.tile_pool(name="qk", bufs=2) as qk, \
                 tc.tile_pool(name="rt", bufs=1) as rt, \
                 tc.tile_pool(name="tp", bufs=2) as tp, \
                 tc.tile_pool(name="ptp", bufs=2) as ptp, \
                 tc.tile_pool(name="ex", bufs=2) as exp_pool, \
                 tc.tile_pool(name="ob", bufs=2) as obp, \
                 tc.tile_pool(name="ps_s", bufs=4, space="PSUM") as ps_s, \
                 tc.tile_pool(name="ps_d", bufs=2, space="PSUM") as ps_d, \
                 tc.tile_pool(name="ps_o", bufs=2, space="PSUM") as ps_o:
                def rope_load(SRC, c0, roped, which, pos0=None):
                    """Load [128, HEADS, WIN] token window at c0 from SRC
                    (head-major scratch), apply RoPE into `roped` (f32r).
                    pos0: column into COS/SINS (halo coords); default c0.
                    cos/sin slices are DMA-replicated x4 so the DVE ops run
                    on [128, 4*WIN] four-head groups."""
                    if pos0 is None:
                        pos0 = c0
                    raw = rt.tile([128, HEADS, WIN], f32, name=f"raw{which}")
                    rot = rt.tile([128, HEADS, WIN], f32, name=f"rot{which}")
                    sl = SRC[:, :, c0:c0 + WIN]
                    nc.sync.dma_start(raw[:], sl.rearrange("h d w -> d h w"))
                    nc.sync.dma_start(rot[0:64], sl[:, 64:128, :].rearrange("h d w -> d h w"))
                    nc.sync.dma_start(rot[64:128], sl[:, 0:64, :].rearrange("h d w -> d h w"))
                    cos4 = tp.tile([128, 4, WIN], f32, name="cos4")
                    sin4 = tp.tile([128, 4, WIN], f32, name="sin4")
                    for g in range(4):
                        nc.sync.dma_start(cos4[:, g], COS[:, pos0:pos0 + WIN])
                        nc.sync.dma_start(sin4[:, g], SINS[:, pos0:pos0 + WIN])
                    for g in range(4):
                        gs = slice(g * 4, (g + 1) * 4)
                        tmp = tp.tile([128, 4, WIN], f32, name="tmp")
                        nc.vector.tensor_mul(tmp[:], rot[:, gs], sin4[:])
                        nc.vector.tensor_mul(roped[:, gs], raw[:, gs], cos4[:])
                        nc.vector.tensor_add(roped[:, gs], roped[:, gs], tmp[:])

                kt_prev = qk.tile([128, HEADS, WIN], f32r, name="kt")
                rope_load(KTS, 0, kt_prev, "k")
                v_prev = qk.tile([128, 2, DIMS], bf16, name="v")
                nc.sync.dma_start(
                    v_prev[:], VS[0:WIN].rearrange("(tb p) c -> p tb c", p=128))

                for c in range(NC_):
                    kt_cur = qk.tile([128, HEADS, WIN], f32r, name="kt")
                    rope_load(KTS, WIN + c * WIN, kt_cur, "k")
                    v_cur = qk.tile([128, 2, DIMS], bf16, name="v")
                    nc.sync.dma_start(
                        v_cur[:], VS[WIN + c * WIN:WIN + (c + 1) * WIN]
                        .rearrange("(tb p) c -> p tb c", p=128))
                    qt = qk.tile([128, HEADS, WIN], f32r, name="qt")
                    rope_load(QTS, c * WIN, qt, "q", pos0=WIN + c * WIN)

                    kts = [kt_prev, kt_prev, kt_cur, kt_cur]
                    vs = [v_prev, v_prev, v_cur, v_cur]
                    W2 = 2 * WIN
                    for h0 in range(0, HEADS, 2):
                        # per head-pair psums: denominator and O^T share
                        # [128, 512] banks (head h0 left, h0+1 right)
                        pd = ps_d.tile([128, W2], f32, name="pd")
                        po = ps_o.tile([128, W2], f32, name="po")
                        pts2 = []
                        for h in (h0, h0 + 1):
                            # scores: kb0|kb1 pair and kb2|kb3 pair in one bank
                            pts = []
                            for pr in range(2):
                                ps = ps_s.tile([128, W2], f32, name="ps")
                                for kb2 in range(2):
                                    kb = pr * 2 + kb2
                                    nc.tensor.matmul(
                                        ps[:, kb2 * WIN:(kb2 + 1) * WIN],
                                        kts[kb][:, h, (kb % 2) * 128:(kb % 2) * 128 + 128],
                                        qt[:, h], start=True, stop=True)
                                pb = ptp.tile([128, W2], bf16, name=f"pt{pr}")
                                if pr == 0:
                                    if c == 0:
                                        nc.scalar.activation(pb[:], ps[:], AF.Exp,
                                                             bias=pgate[:], scale=ISQ)
                                    else:
                                        nc.scalar.activation(pb[:], ps[:], AF.Exp,
                                                             scale=ISQ)
                                else:
                                    ex = exp_pool.tile([128, W2], f32, name="ex")
                                    nc.scalar.activation(ex[:], ps[:], AF.Exp,
                                                         scale=ISQ)
                                    nc.vector.tensor_mul(pb[:], ex[:], tri23[:])
                                pts.append(pb)
                            pts2.append(pts)

                        for i, h in enumerate((h0, h0 + 1)):
                            sl = slice(i * WIN, (i + 1) * WIN)
                            for kb in range(4):
                                pb = pts2[i][kb // 2][:, (kb % 2) * WIN:(kb % 2 + 1) * WIN]
                                nc.tensor.matmul(pd[:, sl], onesm[:], pb,
                                                 start=(kb == 0), stop=(kb == 3))
                            for kb in range(4):
                                pb = pts2[i][kb // 2][:, (kb % 2) * WIN:(kb % 2 + 1) * WIN]
                                nc.tensor.matmul(
                                    po[:, sl], vs[kb][:, kb % 2, h * 128:(h + 1) * 128],
                                    pb, start=(kb == 0), stop=(kb == 3))
                        rb = obp.tile([128, W2], f32, name="rb")
                        with nc.allow_low_precision("softmax denominator"):
                            nc.vector.reciprocal(rb[:], pd[:])
                        ot = obp.tile([128, W2], f32r, name="ot")
                        nc.vector.tensor_mul(ot[:], po[:], rb[:])
                        nc.sync.dma_start(OTS[h0][:, c * WIN:(c + 1) * WIN],
                                          ot[:, 0:WIN])
                        nc.sync.dma_start(OTS[h0 + 1][:, c * WIN:(c + 1) * WIN],
                                          ot[:, WIN:W2])
                    kt_prev, v_prev = kt_cur, v_cur

            # ---------------- P3: output projection ----------------
            if 3 in phases:
              with tc.tile_pool(name="p3", bufs=1) as p3, \
                 tc.tile_pool(name="otp", bufs=2) as otp, \
                 tc.tile_pool(name="so3", bufs=6) as so3, \
                 tc.tile_pool(name="pp3", bufs=8, space="PSUM") as pp3:
                wo = p3.tile([128, IB, DIMS], f32r)
                nc.sync.dma_start(wo[:], WO.rearrange("(ib p) d -> p ib d", p=128))
                for tt in range(T // 128):
                    ots = otp.tile([128, HEADS, 128], f32r, name="ots")
                    nc.sync.dma_start(
                        ots[:], OTS[:, :, tt * 128:(tt + 1) * 128]
                        .rearrange("h d w -> d h w"))
                    for nt in range(4):
                        ps = pp3.tile([128, 512], f32, name="pp3")
                        for h in range(HEADS):
                            nc.tensor.matmul(
                                ps[:], ots[:, h], wo[:, h, nt * 512:(nt + 1) * 512],
                                start=(h == 0), stop=(h == HEADS - 1))
                        so = so3.tile([128, 512], f32, name="so")
                        nc.scalar.copy(so[:], ps[:])
                        nc.sync.dma_start(
                            OUT[tt * 128:(tt + 1) * 128,
                                nt * 512:(nt + 1) * 512], so[:])
    return nc


def _host_inputs(hidden_states, Wq, Wk, Wv, Wo, T):
    """Build the 8 per-core input maps."""
    TH = T + WIN
    inv_freq = 1.0 / (THETA ** (np.arange(0, HD, 2, dtype=np.float32) / HD))

    qq = np.arange(WIN)[None, :]
    kk = np.arange(128)[:, None]
    tri23 = np.concatenate([(qq >= kk), (qq >= kk + 128)], 1).astype(np.float32)
    onesm_bf = np.ones((128, 128), ml_dtypes.bfloat16)

    Wq, Wk, Wv, Wo = (np.ascontiguousarray(w, np.float32) for w in (Wq, Wk, Wv, Wo))
    in_maps = []
    for core in range(8):
        b, sh = divmod(core, NSH)
        t0 = sh * T
        hs = np.zeros((TH, DIMS), np.float32)
        lo = max(0, t0 - WIN)
        hs[WIN - (t0 - lo):] = hidden_states[b, lo:t0 + T]
        hT = np.ascontiguousarray(hs.T)

        pos = np.arange(t0 - WIN, t0 + T, dtype=np.float32)
        f = np.outer(inv_freq, pos)                      # [64, TH]
        cos = np.concatenate([np.cos(f), np.cos(f)], 0)  # [128, TH]
        sin = np.sin(f)
        sins = np.concatenate([-sin, sin], 0)
        pg = np.full((128, 1), -1e30 if sh == 0 else 0.0, np.float32)
        in_maps.append({
            "HT": hT, "WQ": Wq, "WK": Wk, "WV": Wv, "WO": Wo,
            "COS": cos.astype(np.float32), "SINS": sins.astype(np.float32),
            "TRI23": tri23, "PGATE": pg, "ONESM": onesm_bf,
        })
    return in_maps


_CACHE = {}


def run(hidden_states, Wq, Wk, Wv, Wo, T=S // NSH, **spmd_kwargs):
    key = T
    if key not in _CACHE:
        nc = bacc.Bacc(None)
        build(nc, T)
        nc.finalize()
        _CACHE[key] = nc
    nc = _CACHE[key]
    in_maps = _host_inputs(hidden_states, Wq, Wk, Wv, Wo, T)
    res = run_bass_kernel_spmd(nc, in_maps, core_ids=list(range(8)), **spmd_kwargs)
    outs = [res.results[i]["OUT"] for i in range(8)]
    full = np.empty((B, NSH * T, DIMS), np.float32)
    for core in range(8):
        b, sh = divmod(core, NSH)
        full[b, sh * T:(sh + 1) * T] = outs[core]
    return full, res


def kernel(hidden_states, Wq, Wk, Wv, Wo):
    out, _ = run(np.asarray(hidden_states), Wq, Wk, Wv, Wo)
    return out


# revision 19
# speedup vs baseline: 8925.1156x; 8925.1156x over previous
"""Block sliding-window attention on 8 TRN2 NeuronCores.

Sharding: sequence-parallel. 8 shards = (batch b in {0,1}) x (quarter s in
0..3); each core owns 2048 consecutive tokens of one batch plus a 256-token
K/V halo from the previous quarter (zeros + -inf gate for the first quarter).
No collectives: each core computes its tokens' full output rows.

Per-core pipeline (all matmuls in float32r: full PE rate, ~1e-4 rounding):
  P1: QT/KT = W^T @ hiddenT (head-transposed layout, raw), V = hidden @ Wv
      (natural layout), all staged through DRAM scratch.
  P2: per 256-token chunk: RoPE on Q/K (rot-half via partition-offset DMA
      reload + pre-signed sin), then per head: S^T = K Q^T per 128-key block,
      exp on ACT (scale=1/sqrt(128), -1e30 bias gates the no-previous case),
      0/1 triangular mask multiply on DVE (also retypes to f32r), denominator
      via all-ones matmul (broadcasts across partitions), O^T = V^T P^T,
      normalize with DVE reciprocal.
  P3: out = sum_h O_h @ Wo_h, accumulated over all 16 head blocks in PSUM.
"""
import sys

try:
    import concourse  # noqa: F401
except ImportError:
    sys.path.insert(0, '/opt/trn_rl_repo')

import ml_dtypes
import numpy as np

import concourse.bacc as bacc
import concourse.mybir as mybir
import concourse.tile as tile
from concourse.bass_utils import run_bass_kernel_spmd

f32 = mybir.dt.float32
f32r = mybir.dt.float32r
AF = mybir.ActivationFunctionType
bf16 = mybir.dt.bfloat16

DIMS = 2048
HEADS = 16
HD = 128           # head dim
WIN = 256          # window / chunk
B, S = 2, 8192
NSH = 4            # seq shards per batch
THETA = 10000.0
ISQ = float(1.0 / np.sqrt(HD))
IB = DIMS // 128   # 16 input-dim blocks


def build(nc, T, phases=(1, 2, 3)):
    """Emit the per-core program. T = local tokens (multiple of 512)."""
    TH = T + WIN                      # with halo
    NC_ = T // WIN                    # chunks
    HT = nc.dram_tensor("HT", [DIMS, TH], f32r, kind="ExternalInput")
    WQ = nc.dram_tensor("WQ", [DIMS, DIMS], f32r, kind="ExternalInput")
    WK = nc.dram_tensor("WK", [DIMS, DIMS], f32r, kind="ExternalInput")
    WV = nc.dram_tensor("WV", [DIMS, DIMS], f32r, kind="ExternalInput")
    WO = nc.dram_tensor("WO", [DIMS, DIMS], f32r, kind="ExternalInput")
    COS = nc.dram_tensor("COS", [HD, TH], f32, kind="ExternalInput")
    SINS = nc.dram_tensor("SINS", [HD, TH], f32, kind="ExternalInput")
    TRI23 = nc.dram_tensor("TRI23", [128, 2 * WIN], f32, kind="ExternalInput")
    PGATE = nc.dram_tensor("PGATE", [128, 1], f32, kind="ExternalInput")
    ONESM = nc.dram_tensor("ONESM", [128, 128], bf16, kind="ExternalInput")
    OUT = nc.dram_tensor("OUT", [T, DIMS], f32, kind="ExternalOutput")

    QTS = nc.dram_tensor("QTS", [HEADS, HD, T], bf16)    # raw (pre-RoPE) Q^T
    KTS = nc.dram_tensor("KTS", [HEADS, HD, TH], bf16)   # raw K^T (with halo)
    VS = nc.dram_tensor("VS", [TH, DIMS], bf16)         # V natural
    OTS = nc.dram_tensor("OTS", [HEADS, HD, T], f32r)   # normalized O^T

    def tok_tiles(n):
        out, a = [], 0
        while a < n:
            w = min(512, n - a)
            out.append((a, w))
            a += w
        return out

    with tile.TileContext(nc) as tc:
        with tc.tile_pool(name="cst", bufs=1) as cst:
            tri23 = cst.tile([128, 2 * WIN], f32)
            pgate = cst.tile([128, 1], f32)
            onesm = cst.tile([128, 128], bf16)
            nc.sync.dma_start(tri23[:], TRI23[:])
            nc.sync.dma_start(pgate[:], PGATE[:])
            nc.sync.dma_start(onesm[:], ONESM[:])

            # ---------------- P1: projections ----------------
            if 1 in phases:
              with tc.tile_pool(name="p1", bufs=1) as p1, \
                 tc.tile_pool(name="wp", bufs=8) as wp, \
                 tc.tile_pool(name="st", bufs=6) as st, \
                 tc.tile_pool(name="pp", bufs=8, space="PSUM") as pp:
                ht = p1.tile([128, IB, TH], f32r)
                nc.sync.dma_start(ht[:], HT.rearrange("(ib p) t -> p ib t", p=128))

                # QT / KT: lhsT = W block [128in, 128out], rhs = hT
                for W_, DST, t0, tlen in ((WQ, QTS, WIN, T), (WK, KTS, 0, TH)):
                    for ob in range(HEADS):
                        tts = tok_tiles(tlen)
                        psums = [pp.tile([128, 512], f32, name="pp") for _ in tts]
                        for ib in range(IB):
                            wt = wp.tile([128, 128], f32r, name="w")
                            nc.sync.dma_start(
                                wt[:], W_[ib * 128:(ib + 1) * 128,
                                          ob * 128:(ob + 1) * 128])
                            for ti, (a, w) in enumerate(tts):
                                nc.tensor.matmul(
                                    psums[ti][:, :w], wt[:],
                                    ht[:, ib, t0 + a:t0 + a + w],
                                    start=(ib == 0), stop=(ib == IB - 1))
                        for ti, (a, w) in enumerate(tts):
                            so = st.tile([128, 512], bf16, name="st")
                            nc.scalar.copy(so[:, :w], psums[ti][:, :w])
                            nc.sync.dma_start(DST[ob][:, a:a + w], so[:, :w])

                # V natural: lhsT = hT block [128in, 128tok], rhs = Wv rows
                NTB = TH // 128
                for tb0 in range(0, NTB, 4):
                    tbs = list(range(tb0, min(tb0 + 4, NTB)))
                    for og in range(4):
                        psums = {}
                        for ib in range(IB):
                            wt = wp.tile([128, 512], f32r, name="wv")
                            nc.sync.dma_start(
                                wt[:], WV[ib * 128:(ib + 1) * 128,
                                          og * 512:(og + 1) * 512])
                            for tb in tbs:
                                if ib == 0:
                                    psums[tb] = pp.tile([128, 512], f32, name="pp")
                                nc.tensor.matmul(
                                    psums[tb][:],
                                    ht[:, ib, tb * 128:(tb + 1) * 128], wt[:],
                                    start=(ib == 0), stop=(ib == IB - 1))
                        for tb in tbs:
                            so = st.tile([128, 512], bf16, name="stv")
                            nc.vector.tensor_copy(so[:], psums[tb][:])
                            nc.sync.dma_start(
                                VS[tb * 128:(tb + 1) * 128,
                                   og * 512:(og + 1) * 512], so[:])

            # ---------------- P2: attention ----------------
            if 2 in phases:
              with tc.tile_pool(name="qk", bufs=2) as qk, \
                 tc.tile_pool(name="rt", bufs=1) as rt, \
                 tc.tile_pool(name="tp", bufs=2) as tp, \
                 tc.tile_pool(name="ptp", bufs=2) as ptp, \
                 tc.tile_pool(name="ex", bufs=2) as exp_pool, \
                 tc.tile_pool(name="ob", bufs=2) as obp, \
                 tc.tile_pool(name="ps_s", bufs=4, space="PSUM") as ps_s, \
                 tc.tile_pool(name="ps_d", bufs=2, space="PSUM") as ps_d, \
                 tc.tile_pool(name="ps_o", bufs=2, space="PSUM") as ps_o:
                def rope_load(SRC, c0, roped, which, pos0=None):
                    """Load [128, HEADS, WIN] token window at c0 from SRC
                    (head-major scratch), apply RoPE into `roped` (f32r).
                    pos0: column into COS/SINS (halo coords); default c0.
                    cos/sin slices are DMA-replicated x4 so the DVE ops run
                    on [128, 4*WIN] four-head groups."""
                    if pos0 is None:
                        pos0 = c0
                    raw = rt.tile([128, HEADS, WIN], bf16, name=f"raw{which}")
                    rot = rt.tile([128, HEADS, WIN], bf16, name=f"rot{which}")
                    sl = SRC[:, :, c0:c0 + WIN]
                    nc.sync.dma_start(raw[:], sl.rearrange("h d w -> d h w"))
                    nc.sync.dma_start(rot[0:64], sl[:, 64:128, :].rearrange("h d w -> d h w"))
                    nc.sync.dma_start(rot[64:128], sl[:, 0:64, :].rearrange("h d w -> d h w"))
                    cos4 = tp.tile([128, 4, WIN], f32, name="cos4")
                    sin4 = tp.tile([128, 4, WIN], f32, name="sin4")
                    for g in range(4):
                        nc.sync.dma_start(cos4[:, g], COS[:, pos0:pos0 + WIN])
                        nc.sync.dma_start(sin4[:, g], SINS[:, pos0:pos0 + WIN])
                    for g in range(4):
                        gs = slice(g * 4, (g + 1) * 4)
                        tmp = tp.tile([128, 4, WIN], bf16, name="tmp")
                        nc.vector.tensor_mul(tmp[:], rot[:, gs], sin4[:])
                        nc.vector.tensor_mul(roped[:, gs], raw[:, gs], cos4[:])
                        nc.vector.tensor_add(roped[:, gs], roped[:, gs], tmp[:])

                kt_prev = qk.tile([128, HEADS, WIN], bf16, name="kt")
                rope_load(KTS, 0, kt_prev, "k")
                v_prev = qk.tile([128, 2, DIMS], bf16, name="v")
                nc.sync.dma_start(
                    v_prev[:], VS[0:WIN].rearrange("(tb p) c -> p tb c", p=128))

                for c in range(NC_):
                    kt_cur = qk.tile([128, HEADS, WIN], bf16, name="kt")
                    rope_load(KTS, WIN + c * WIN, kt_cur, "k")
                    v_cur = qk.tile([128, 2, DIMS], bf16, name="v")
                    nc.sync.dma_start(
                        v_cur[:], VS[WIN + c * WIN:WIN + (c + 1) * WIN]
                        .rearrange("(tb p) c -> p tb c", p=128))
                    qt = qk.tile([128, HEADS, WIN], bf16, name="qt")
                    rope_load(QTS, c * WIN, qt, "q", pos0=WIN + c * WIN)

                    kts = [kt_prev, kt_prev, kt_cur, kt_cur]
                    vs = [v_prev, v_prev, v_cur, v_cur]
                    W2 = 2 * WIN
                    for h0 in range(0, HEADS, 2):
                        # per head-pair psums: denominator and O^T share
                        # [128, 512] banks (head h0 left, h0+1 right)
                        pd = ps_d.tile([128, W2], f32, name="pd")
                        po = ps_o.tile([128, W2], f32, name="po")
                        pts2 = []
                        for h in (h0, h0 + 1):
                            # scores: kb0|kb1 pair and kb2|kb3 pair in one bank
                            pts = []
                            for pr in range(2):
                                ps = ps_s.tile([128, W2], f32, name="ps")
                                for kb2 in range(2):
                                    kb = pr * 2 + kb2
                                    nc.tensor.matmul(
                                        ps[:, kb2 * WIN:(kb2 + 1) * WIN],
                                        kts[kb][:, h, (kb % 2) * 128:(kb % 2) * 128 + 128],
                                        qt[:, h], start=True, stop=True)
                                pb = ptp.tile([128, W2], bf16, name=f"pt{pr}")
                                if pr == 0:
                                    if c == 0:
                                        nc.scalar.activation(pb[:], ps[:], AF.Exp,
                                                             bias=pgate[:], scale=ISQ)
                                    else:
                                        nc.scalar.activation(pb[:], ps[:], AF.Exp,
                                                             scale=ISQ)
                                else:
                                    ex = exp_pool.tile([128, W2], f32, name="ex")
                                    nc.scalar.activation(ex[:], ps[:], AF.Exp,
                                                         scale=ISQ)
                                    nc.vector.tensor_mul(pb[:], ex[:], tri23[:])
                                pts.append(pb)
                            pts2.append(pts)

                        for i, h in enumerate((h0, h0 + 1)):
                            sl = slice(i * WIN, (i + 1) * WIN)
                            for kb in range(4):
                                pb = pts2[i][kb // 2][:, (kb % 2) * WIN:(kb % 2 + 1) * WIN]
                                nc.tensor.matmul(pd[:, sl], onesm[:], pb,
                                                 start=(kb == 0), stop=(kb == 3))
                            for kb in range(4):
                                pb = pts2[i][kb // 2][:, (kb % 2) * WIN:(kb % 2 + 1) * WIN]
                                nc.tensor.matmul(
                                    po[:, sl], vs[kb][:, kb % 2, h * 128:(h + 1) * 128],
                                    pb, start=(kb == 0), stop=(kb == 3))
                        rb = obp.tile([128, W2], f32, name="rb")
                        with nc.allow_low_precision("softmax denominator"):
                            nc.vector.reciprocal(rb[:], pd[:])
                        ot = obp.tile([128, W2], f32r, name="ot")
                        nc.vector.tensor_mul(ot[:], po[:], rb[:])
                        nc.sync.dma_start(OTS[h0][:, c * WIN:(c + 1) * WIN],
                                          ot[:, 0:WIN])
                        nc.sync.dma_start(OTS[h0 + 1][:, c * WIN:(c + 1) * WIN],
                                          ot[:, WIN:W2])
                    kt_prev, v_prev = kt_cur, v_cur

            # ---------------- P3: output projection ----------------
            if 3 in phases:
              with tc.tile_pool(name="p3", bufs=1) as p3, \
                 tc.tile_pool(name="otp", bufs=2) as otp, \
                 tc.tile_pool(name="so3", bufs=6) as so3, \
                 tc.tile_pool(name="pp3", bufs=8, space="PSUM") as pp3:
                wo = p3.tile([128, IB, DIMS], f32r)
                nc.sync.dma_start(wo[:], WO.rearrange("(ib p) d -> p ib d", p=128))
                for tt in range(T // 128):
                    ots = otp.tile([128, HEADS, 128], f32r, name="ots")
                    nc.sync.dma_start(
                        ots[:], OTS[:, :, tt * 128:(tt + 1) * 128]
                        .rearrange("h d w -> d h w"))
                    for nt in range(4):
                        ps = pp3.tile([128, 512], f32, name="pp3")
                        for h in range(HEADS):
                            nc.tensor.matmul(
                                ps[:], ots[:, h], wo[:, h, nt * 512:(nt + 1) * 512],
                                start=(h == 0), stop=(h == HEADS - 1))
                        so = so3.tile([128, 512], f32, name="so")
                        nc.scalar.copy(so[:], ps[:])
                        nc.sync.dma_start(
                            OUT[tt * 128:(tt + 1) * 128,
                                nt * 512:(nt + 1) * 512], so[:])
    return nc


def _host_inputs(hidden_states, Wq, Wk, Wv, Wo, T):
    """Build the 8 per-core input maps."""
    TH = T + WIN
    inv_freq = 1.0 / (THETA ** (np.arange(0, HD, 2, dtype=np.float32) / HD))

    qq = np.arange(WIN)[None, :]
    kk = np.arange(128)[:, None]
    tri23 = np.concatenate([(qq >= kk), (qq >= kk + 128)], 1).astype(np.float32)
    onesm_bf = np.ones((128, 128), ml_dtypes.bfloat16)

    Wq, Wk, Wv, Wo = (np.ascontiguousarray(w, np.float32) for w in (Wq, Wk, Wv, Wo))
    in_maps = []
    for core in range(8):
        b, sh = divmod(core, NSH)
        t0 = sh * T
        hs = np.zeros((TH, DIMS), np.float32)
        lo = max(0, t0 - WIN)
        hs[WIN - (t0 - lo):] = hidden_states[b, lo:t0 + T]
        hT = np.ascontiguousarray(hs.T)

        pos = np.arange(t0 - WIN, t0 + T, dtype=np.float32)
        f = np.outer(inv_freq, pos)                      # [64, TH]
        cos = np.concatenate([np.cos(f), np.cos(f)], 0)  # [128, TH]
        sin = np.sin(f)
        sins = np.concatenate([-sin, sin], 0)
        pg = np.full((128, 1), -1e30 if sh == 0 else 0.0, np.float32)
        in_maps.append({
            "HT": hT, "WQ": Wq, "WK": Wk, "WV": Wv, "WO": Wo,
            "COS": cos.astype(np.float32), "SINS": sins.astype(np.float32),
            "TRI23": tri23, "PGATE": pg, "ONESM": onesm_bf,
        })
    return in_maps


_CACHE = {}


def run(hidden_states, Wq, Wk, Wv, Wo, T=S // NSH, **spmd_kwargs):
    key = T
    if key not in _CACHE:
        nc = bacc.Bacc(None)
        build(nc, T)
        nc.finalize()
        _CACHE[key] = nc
    nc = _CACHE[key]
    in_maps = _host_inputs(hidden_states, Wq, Wk, Wv, Wo, T)
    res = run_bass_kernel_spmd(nc, in_maps, core_ids=list(range(8)), **spmd_kwargs)
    outs = [res.results[i]["OUT"] for i in range(8)]
    full = np.empty((B, NSH * T, DIMS), np.float32)
    for core in range(8):
        b, sh = divmod(core, NSH)
        full[b, sh * T:(sh + 1) * T] = outs[core]
    return full, res


def kernel(hidden_states, Wq, Wk, Wv, Wo):
    out, _ = run(np.asarray(hidden_states), Wq, Wk, Wv, Wo)
    return out


# revision 22
# speedup vs baseline: 9132.7155x; 1.0233x over previous
"""Block sliding-window attention on 8 TRN2 NeuronCores.

Sharding: sequence-parallel. 8 shards = (batch b in {0,1}) x (quarter s in
0..3); each core owns 2048 consecutive tokens of one batch plus a 256-token
K/V halo from the previous quarter (zeros + -inf gate for the first quarter).
No collectives: each core computes its tokens' full output rows.

Per-core pipeline (all matmuls in float32r: full PE rate, ~1e-4 rounding):
  P1: QT/KT = W^T @ hiddenT (head-transposed layout, raw), V = hidden @ Wv
      (natural layout), all staged through DRAM scratch.
  P2: per 256-token chunk: RoPE on Q/K (rot-half via partition-offset DMA
      reload + pre-signed sin), then per head: S^T = K Q^T per 128-key block,
      exp on ACT (scale=1/sqrt(128), -1e30 bias gates the no-previous case),
      0/1 triangular mask multiply on DVE (also retypes to f32r), denominator
      via all-ones matmul (broadcasts across partitions), O^T = V^T P^T,
      normalize with DVE reciprocal.
  P3: out = sum_h O_h @ Wo_h, accumulated over all 16 head blocks in PSUM.
"""
import sys

try:
    import concourse  # noqa: F401
except ImportError:
    sys.path.insert(0, '/opt/trn_rl_repo')

import ml_dtypes
import numpy as np

import concourse.bacc as bacc
import concourse.mybir as mybir
import concourse.tile as tile
from concourse.bass_utils import run_bass_kernel_spmd

f32 = mybir.dt.float32
f32r = mybir.dt.float32r
AF = mybir.ActivationFunctionType
bf16 = mybir.dt.bfloat16

DIMS = 2048
HEADS = 16
HD = 128           # head dim
WIN = 256          # window / chunk
B, S = 2, 8192
NSH = 4            # seq shards per batch
THETA = 10000.0
ISQ = float(1.0 / np.sqrt(HD))
IB = DIMS // 128   # 16 input-dim blocks


def build(nc, T, phases=(1, 2, 3)):
    """Emit the per-core program. T = local tokens (multiple of 512)."""
    TH = T + WIN                      # with halo
    NC_ = T // WIN                    # chunks
    HT = nc.dram_tensor("HT", [DIMS, TH], f32r, kind="ExternalInput")
    WQ = nc.dram_tensor("WQ", [DIMS, DIMS], f32r, kind="ExternalInput")
    WK = nc.dram_tensor("WK", [DIMS, DIMS], f32r, kind="ExternalInput")
    WV = nc.dram_tensor("WV", [DIMS, DIMS], f32r, kind="ExternalInput")
    WO = nc.dram_tensor("WO", [DIMS, DIMS], f32r, kind="ExternalInput")
    COS = nc.dram_tensor("COS", [HD, TH], f32, kind="ExternalInput")
    SINS = nc.dram_tensor("SINS", [HD, TH], f32, kind="ExternalInput")
    TRI23 = nc.dram_tensor("TRI23", [128, 2 * WIN], bf16, kind="ExternalInput")
    PGATE = nc.dram_tensor("PGATE", [128, 1], f32, kind="ExternalInput")
    ONESM = nc.dram_tensor("ONESM", [128, 128], bf16, kind="ExternalInput")
    OUT = nc.dram_tensor("OUT", [T, DIMS], f32, kind="ExternalOutput")

    QTS = nc.dram_tensor("QTS", [HEADS, HD, T], bf16)    # raw (pre-RoPE) Q^T
    KTS = nc.dram_tensor("KTS", [HEADS, HD, TH], bf16)   # raw K^T (with halo)
    VS = nc.dram_tensor("VS", [TH, DIMS], bf16)         # V natural
    OTS = nc.dram_tensor("OTS", [HEADS, HD, T], f32r)   # normalized O^T

    def tok_tiles(n):
        out, a = [], 0
        while a < n:
            w = min(512, n - a)
            out.append((a, w))
            a += w
        return out

    with tile.TileContext(nc) as tc:
        with tc.tile_pool(name="cst", bufs=1) as cst:
            tri23 = cst.tile([128, 2 * WIN], bf16)
            pgate = cst.tile([128, 1], f32)
            onesm = cst.tile([128, 128], bf16)
            nc.sync.dma_start(tri23[:], TRI23[:])
            nc.sync.dma_start(pgate[:], PGATE[:])
            nc.sync.dma_start(onesm[:], ONESM[:])

            # ---------------- P1: projections ----------------
            if 1 in phases:
              with tc.tile_pool(name="p1", bufs=1) as p1, \
                 tc.tile_pool(name="wp", bufs=10) as wp, \
                 tc.tile_pool(name="st", bufs=8) as st, \
                 tc.tile_pool(name="pp", bufs=8, space="PSUM") as pp:
                ht = p1.tile([128, IB, TH], f32r)
                nc.sync.dma_start(ht[:], HT.rearrange("(ib p) t -> p ib t", p=128))

                # QT / KT: lhsT = W block [128in, 128out], rhs = hT
                for W_, DST, t0, tlen in ((WQ, QTS, WIN, T), (WK, KTS, 0, TH)):
                    for ob in range(HEADS):
                        tts = tok_tiles(tlen)
                        psums = [pp.tile([128, 512], f32, name="pp") for _ in tts]
                        for ib in range(IB):
                            wt = wp.tile([128, 128], f32r, name="w")
                            nc.sync.dma_start(
                                wt[:], W_[ib * 128:(ib + 1) * 128,
                                          ob * 128:(ob + 1) * 128])
                            for ti, (a, w) in enumerate(tts):
                                nc.tensor.matmul(
                                    psums[ti][:, :w], wt[:],
                                    ht[:, ib, t0 + a:t0 + a + w],
                                    start=(ib == 0), stop=(ib == IB - 1))
                        for ti, (a, w) in enumerate(tts):
                            so = st.tile([128, 512], bf16, name="st")
                            nc.scalar.copy(so[:, :w], psums[ti][:, :w])
                            nc.sync.dma_start(DST[ob][:, a:a + w], so[:, :w])

                # V natural: lhsT = hT block [128in, 128tok], rhs = Wv rows
                NTB = TH // 128
                for tb0 in range(0, NTB, 4):
                    tbs = list(range(tb0, min(tb0 + 4, NTB)))
                    for og in range(4):
                        psums = {}
                        for ib in range(IB):
                            wt = wp.tile([128, 512], f32r, name="wv")
                            nc.sync.dma_start(
                                wt[:], WV[ib * 128:(ib + 1) * 128,
                                          og * 512:(og + 1) * 512])
                            for tb in tbs:
                                if ib == 0:
                                    psums[tb] = pp.tile([128, 512], f32, name="pp")
                                nc.tensor.matmul(
                                    psums[tb][:],
                                    ht[:, ib, tb * 128:(tb + 1) * 128], wt[:],
                                    start=(ib == 0), stop=(ib == IB - 1))
                        for tb in tbs:
                            so = st.tile([128, 512], bf16, name="stv")
                            nc.vector.tensor_copy(so[:], psums[tb][:])
                            nc.sync.dma_start(
                                VS[tb * 128:(tb + 1) * 128,
                                   og * 512:(og + 1) * 512], so[:])

            # ---------------- P2: attention ----------------
            if 2 in phases:
              with tc.tile_pool(name="qk", bufs=2) as qk, \
                 tc.tile_pool(name="rt", bufs=1) as rt, \
                 tc.tile_pool(name="tp", bufs=3) as tp, \
                 tc.tile_pool(name="ptp", bufs=2) as ptp, \
                 tc.tile_pool(name="ex", bufs=2) as exp_pool, \
                 tc.tile_pool(name="ob", bufs=2) as obp, \
                 tc.tile_pool(name="ps_s", bufs=4, space="PSUM") as ps_s, \
                 tc.tile_pool(name="ps_d", bufs=2, space="PSUM") as ps_d, \
                 tc.tile_pool(name="ps_o", bufs=2, space="PSUM") as ps_o:
                def rope_load(SRC, c0, roped, which, pos0=None):
                    """Load [128, HEADS, WIN] token window at c0 from SRC
                    (head-major scratch), apply RoPE into `roped` (f32r).
                    pos0: column into COS/SINS (halo coords); default c0.
                    cos/sin slices are DMA-replicated x4 so the DVE ops run
                    on [128, 4*WIN] four-head groups."""
                    if pos0 is None:
                        pos0 = c0
                    raw = rt.tile([128, HEADS, WIN], bf16, name=f"raw{which}")
                    rot = rt.tile([128, HEADS, WIN], bf16, name=f"rot{which}")
                    sl = SRC[:, :, c0:c0 + WIN]
                    nc.sync.dma_start(raw[:], sl.rearrange("h d w -> d h w"))
                    nc.sync.dma_start(rot[0:64], sl[:, 64:128, :].rearrange("h d w -> d h w"))
                    nc.sync.dma_start(rot[64:128], sl[:, 0:64, :].rearrange("h d w -> d h w"))
                    cos4 = tp.tile([128, 4, WIN], f32, name="cos4")
                    sin4 = tp.tile([128, 4, WIN], f32, name="sin4")
                    for g in range(4):
                        nc.sync.dma_start(cos4[:, g], COS[:, pos0:pos0 + WIN])
                        nc.sync.dma_start(sin4[:, g], SINS[:, pos0:pos0 + WIN])
                    for g in range(4):
                        gs = slice(g * 4, (g + 1) * 4)
                        tmp = tp.tile([128, 4, WIN], bf16, name="tmp")
                        nc.vector.tensor_mul(tmp[:], rot[:, gs], sin4[:])
                        nc.vector.tensor_mul(roped[:, gs], raw[:, gs], cos4[:])
                        nc.vector.tensor_add(roped[:, gs], roped[:, gs], tmp[:])

                kt_prev = qk.tile([128, HEADS, WIN], bf16, name="kt")
                rope_load(KTS, 0, kt_prev, "k")
                v_prev = qk.tile([128, 2, DIMS], bf16, name="v")
                nc.sync.dma_start(
                    v_prev[:], VS[0:WIN].rearrange("(tb p) c -> p tb c", p=128))

                for c in range(NC_):
                    kt_cur = qk.tile([128, HEADS, WIN], bf16, name="kt")
                    rope_load(KTS, WIN + c * WIN, kt_cur, "k")
                    v_cur = qk.tile([128, 2, DIMS], bf16, name="v")
                    nc.sync.dma_start(
                        v_cur[:], VS[WIN + c * WIN:WIN + (c + 1) * WIN]
                        .rearrange("(tb p) c -> p tb c", p=128))
                    qt = qk.tile([128, HEADS, WIN], bf16, name="qt")
                    rope_load(QTS, c * WIN, qt, "q", pos0=WIN + c * WIN)

                    kts = [kt_prev, kt_prev, kt_cur, kt_cur]
                    vs = [v_prev, v_prev, v_cur, v_cur]
                    W2 = 2 * WIN
                    for h0 in range(0, HEADS, 2):
                        # per head-pair psums: denominator and O^T share
                        # [128, 512] banks (head h0 left, h0+1 right)
                        pd = ps_d.tile([128, W2], f32, name="pd")
                        po = ps_o.tile([128, W2], f32, name="po")
                        pts2 = []
                        for h in (h0, h0 + 1):
                            # scores: kb0|kb1 pair and kb2|kb3 pair in one bank
                            pts = []
                            for pr in range(2):
                                ps = ps_s.tile([128, W2], f32, name="ps")
                                for kb2 in range(2):
                                    kb = pr * 2 + kb2
                                    nc.tensor.matmul(
                                        ps[:, kb2 * WIN:(kb2 + 1) * WIN],
                                        kts[kb][:, h, (kb % 2) * 128:(kb % 2) * 128 + 128],
                                        qt[:, h], start=True, stop=True)
                                pb = ptp.tile([128, W2], bf16, name=f"pt{pr}")
                                if pr == 0:
                                    if c == 0:
                                        nc.scalar.activation(pb[:], ps[:], AF.Exp,
                                                             bias=pgate[:], scale=ISQ)
                                    else:
                                        nc.scalar.activation(pb[:], ps[:], AF.Exp,
                                                             scale=ISQ)
                                else:
                                    ex = exp_pool.tile([128, W2], bf16, name="ex")
                                    nc.scalar.activation(ex[:], ps[:], AF.Exp,
                                                         scale=ISQ)
                                    nc.vector.tensor_mul(pb[:], ex[:], tri23[:])
                                pts.append(pb)
                            pts2.append(pts)

                        for i, h in enumerate((h0, h0 + 1)):
                            sl = slice(i * WIN, (i + 1) * WIN)
                            for kb in range(4):
                                pb = pts2[i][kb // 2][:, (kb % 2) * WIN:(kb % 2 + 1) * WIN]
                                nc.tensor.matmul(pd[:, sl], onesm[:], pb,
                                                 start=(kb == 0), stop=(kb == 3))
                            for kb in range(4):
                                pb = pts2[i][kb // 2][:, (kb % 2) * WIN:(kb % 2 + 1) * WIN]
                                nc.tensor.matmul(
                                    po[:, sl], vs[kb][:, kb % 2, h * 128:(h + 1) * 128],
                                    pb, start=(kb == 0), stop=(kb == 3))
                        rb = obp.tile([128, W2], f32, name="rb")
                        with nc.allow_low_precision("softmax denominator"):
                            nc.vector.reciprocal(rb[:], pd[:])
                        ot = obp.tile([128, W2], f32r, name="ot")
                        nc.vector.tensor_mul(ot[:], po[:], rb[:])
                        nc.sync.dma_start(OTS[h0][:, c * WIN:(c + 1) * WIN],
                                          ot[:, 0:WIN])
                        nc.sync.dma_start(OTS[h0 + 1][:, c * WIN:(c + 1) * WIN],
                                          ot[:, WIN:W2])
                    kt_prev, v_prev = kt_cur, v_cur

            # ---------------- P3: output projection ----------------
            if 3 in phases:
              with tc.tile_pool(name="p3", bufs=1) as p3, \
                 tc.tile_pool(name="otp", bufs=3) as otp, \
                 tc.tile_pool(name="so3", bufs=6) as so3, \
                 tc.tile_pool(name="pp3", bufs=8, space="PSUM") as pp3:
                wo = p3.tile([128, IB, DIMS], f32r)
                nc.sync.dma_start(wo[:], WO.rearrange("(ib p) d -> p ib d", p=128))
                for tt in range(T // 128):
                    ots = otp.tile([128, HEADS, 128], f32r, name="ots")
                    nc.sync.dma_start(
                        ots[:], OTS[:, :, tt * 128:(tt + 1) * 128]
                        .rearrange("h d w -> d h w"))
                    for nt in range(4):
                        ps = pp3.tile([128, 512], f32, name="pp3")
                        for h in range(HEADS):
                            nc.tensor.matmul(
                                ps[:], ots[:, h], wo[:, h, nt * 512:(nt + 1) * 512],
                                start=(h == 0), stop=(h == HEADS - 1))
                        so = so3.tile([128, 512], f32, name="so")
                        nc.scalar.copy(so[:], ps[:])
                        nc.sync.dma_start(
                            OUT[tt * 128:(tt + 1) * 128,
                                nt * 512:(nt + 1) * 512], so[:])
    return nc


def _host_inputs(hidden_states, Wq, Wk, Wv, Wo, T):
    """Build the 8 per-core input maps."""
    TH = T + WIN
    inv_freq = 1.0 / (THETA ** (np.arange(0, HD, 2, dtype=np.float32) / HD))

    qq = np.arange(WIN)[None, :]
    kk = np.arange(128)[:, None]
    tri23 = np.concatenate([(qq >= kk), (qq >= kk + 128)], 1).astype(ml_dtypes.bfloat16)
    onesm_bf = np.ones((128, 128), ml_dtypes.bfloat16)

    Wq, Wk, Wv, Wo = (np.ascontiguousarray(w, np.float32) for w in (Wq, Wk, Wv, Wo))
    in_maps = []
    for core in range(8):
        b, sh = divmod(core, NSH)
        t0 = sh * T
        hs = np.zeros((TH, DIMS), np.float32)
        lo = max(0, t0 - WIN)
        hs[WIN - (t0 - lo):] = hidden_states[b, lo:t0 + T]
        hT = np.ascontiguousarray(hs.T)

        pos = np.arange(t0 - WIN, t0 + T, dtype=np.float32)
        f = np.outer(inv_freq, pos)                      # [64, TH]
        cos = np.concatenate([np.cos(f), np.cos(f)], 0)  # [128, TH]
        sin = np.sin(f)
        sins = np.concatenate([-sin, sin], 0)
        pg = np.full((128, 1), -1e30 if sh == 0 else 0.0, np.float32)
        in_maps.append({
            "HT": hT, "WQ": Wq, "WK": Wk, "WV": Wv, "WO": Wo,
            "COS": cos.astype(np.float32), "SINS": sins.astype(np.float32),
            "TRI23": tri23, "PGATE": pg, "ONESM": onesm_bf,
        })
    return in_maps


_CACHE = {}


def run(hidden_states, Wq, Wk, Wv, Wo, T=S // NSH, **spmd_kwargs):
    key = T
    if key not in _CACHE:
        nc = bacc.Bacc(None)
        build(nc, T)
        nc.finalize()
        _CACHE[key] = nc
    nc = _CACHE[key]
    in_maps = _host_inputs(hidden_states, Wq, Wk, Wv, Wo, T)
    res = run_bass_kernel_spmd(nc, in_maps, core_ids=list(range(8)), **spmd_kwargs)
    outs = [res.results[i]["OUT"] for i in range(8)]
    full = np.empty((B, NSH * T, DIMS), np.float32)
    for core in range(8):
        b, sh = divmod(core, NSH)
        full[b, sh * T:(sh + 1) * T] = outs[core]
    return full, res


def kernel(hidden_states, Wq, Wk, Wv, Wo):
    out, _ = run(np.asarray(hidden_states), Wq, Wk, Wv, Wo)
    return out


# revision 26
# speedup vs baseline: 9431.2923x; 1.0327x over previous
"""Block sliding-window attention on 8 TRN2 NeuronCores.

Sharding: sequence-parallel. 8 shards = (batch b in {0,1}) x (quarter s in
0..3); each core owns 2048 consecutive tokens of one batch plus a 256-token
K/V halo from the previous quarter (zeros + -inf gate for the first quarter).
No collectives: each core computes its tokens' full output rows.

Per-core pipeline (all matmuls in float32r: full PE rate, ~1e-4 rounding):
  P1: QT/KT = W^T @ hiddenT (head-transposed layout, raw), V = hidden @ Wv
      (natural layout), all staged through DRAM scratch.
  P2: per 256-token chunk: RoPE on Q/K (rot-half via partition-offset DMA
      reload + pre-signed sin), then per head: S^T = K Q^T per 128-key block,
      exp on ACT (scale=1/sqrt(128), -1e30 bias gates the no-previous case),
      0/1 triangular mask multiply on DVE (also retypes to f32r), denominator
      via all-ones matmul (broadcasts across partitions), O^T = V^T P^T,
      normalize with DVE reciprocal.
  P3: out = sum_h O_h @ Wo_h, accumulated over all 16 head blocks in PSUM.
"""
import sys

try:
    import concourse  # noqa: F401
except ImportError:
    sys.path.insert(0, '/opt/trn_rl_repo')

import ml_dtypes
import numpy as np

import concourse.bacc as bacc
import concourse.mybir as mybir
import concourse.tile as tile
from concourse.bass_utils import run_bass_kernel_spmd

f32 = mybir.dt.float32
f32r = mybir.dt.float32r
AF = mybir.ActivationFunctionType
bf16 = mybir.dt.bfloat16

DIMS = 2048
HEADS = 16
HD = 128           # head dim
WIN = 256          # window / chunk
B, S = 2, 8192
NSH = 4            # seq shards per batch
THETA = 10000.0
ISQ = float(1.0 / np.sqrt(HD))
IB = DIMS // 128   # 16 input-dim blocks


def build(nc, T, phases=(1, 2, 3)):
    """Emit the per-core program. T = local tokens (multiple of 512)."""
    TH = T + WIN                      # with halo
    NC_ = T // WIN                    # chunks
    HT = nc.dram_tensor("HT", [DIMS, TH], f32r, kind="ExternalInput")
    WQ = nc.dram_tensor("WQ", [DIMS, DIMS], f32r, kind="ExternalInput")
    WK = nc.dram_tensor("WK", [DIMS, DIMS], f32r, kind="ExternalInput")
    WV = nc.dram_tensor("WV", [DIMS, DIMS], f32r, kind="ExternalInput")
    WO = nc.dram_tensor("WO", [DIMS, DIMS], f32r, kind="ExternalInput")
    COS = nc.dram_tensor("COS", [HD, TH], f32, kind="ExternalInput")
    SINS = nc.dram_tensor("SINS", [HD, TH], f32, kind="ExternalInput")
    TRI23 = nc.dram_tensor("TRI23", [128, 2 * WIN], bf16, kind="ExternalInput")
    PGATE = nc.dram_tensor("PGATE", [128, 1], f32, kind="ExternalInput")
    ONESM = nc.dram_tensor("ONESM", [128, 128], bf16, kind="ExternalInput")
    OUT = nc.dram_tensor("OUT", [T, DIMS], f32, kind="ExternalOutput")

    QTS = nc.dram_tensor("QTS", [HEADS, HD, T], bf16)    # raw (pre-RoPE) Q^T
    KTS = nc.dram_tensor("KTS", [HEADS, HD, TH], bf16)   # raw K^T (with halo)
    VS = nc.dram_tensor("VS", [TH, DIMS], bf16)         # V natural
    OTS = nc.dram_tensor("OTS", [HEADS, HD, T], f32r)   # normalized O^T

    def tok_tiles(n):
        out, a = [], 0
        while a < n:
            w = min(512, n - a)
            out.append((a, w))
            a += w
        return out

    with tile.TileContext(nc) as tc:
        with tc.tile_pool(name="cst", bufs=1) as cst:
            tri23 = cst.tile([128, 2 * WIN], bf16)
            pgate = cst.tile([128, 1], f32)
            onesm = cst.tile([128, 128], bf16)
            nc.sync.dma_start(tri23[:], TRI23[:])
            nc.sync.dma_start(pgate[:], PGATE[:])
            nc.sync.dma_start(onesm[:], ONESM[:])

            # ---------------- P1: projections ----------------
            if 1 in phases:
              with tc.tile_pool(name="p1", bufs=1) as p1, \
                 tc.tile_pool(name="wp", bufs=10) as wp, \
                 tc.tile_pool(name="st", bufs=8) as st, \
                 tc.tile_pool(name="pp", bufs=8, space="PSUM") as pp:
                ht = p1.tile([128, IB, TH], f32r)
                nc.sync.dma_start(ht[:], HT.rearrange("(ib p) t -> p ib t", p=128))

                # QT / KT: lhsT = W block [128in, 128out], rhs = hT
                for W_, DST, t0, tlen in ((WQ, QTS, WIN, T), (WK, KTS, 0, TH)):
                    for ob in range(HEADS):
                        tts = tok_tiles(tlen)
                        psums = [pp.tile([128, 512], f32, name="pp") for _ in tts]
                        for ib in range(IB):
                            wt = wp.tile([128, 128], f32r, name="w")
                            nc.sync.dma_start(
                                wt[:], W_[ib * 128:(ib + 1) * 128,
                                          ob * 128:(ob + 1) * 128])
                            for ti, (a, w) in enumerate(tts):
                                nc.tensor.matmul(
                                    psums[ti][:, :w], wt[:],
                                    ht[:, ib, t0 + a:t0 + a + w],
                                    start=(ib == 0), stop=(ib == IB - 1))
                        for ti, (a, w) in enumerate(tts):
                            so = st.tile([128, 512], bf16, name="st")
                            nc.scalar.copy(so[:, :w], psums[ti][:, :w])
                            nc.sync.dma_start(DST[ob][:, a:a + w], so[:, :w])

                # V natural: lhsT = hT block [128in, 128tok], rhs = Wv rows
                NTB = TH // 128
                for tb0 in range(0, NTB, 6):
                    tbs = list(range(tb0, min(tb0 + 6, NTB)))
                    for og in range(4):
                        psums = {}
                        for ib in range(IB):
                            wt = wp.tile([128, 512], f32r, name="wv")
                            nc.sync.dma_start(
                                wt[:], WV[ib * 128:(ib + 1) * 128,
                                          og * 512:(og + 1) * 512])
                            for tb in tbs:
                                if ib == 0:
                                    psums[tb] = pp.tile([128, 512], f32, name="pp")
                                nc.tensor.matmul(
                                    psums[tb][:],
                                    ht[:, ib, tb * 128:(tb + 1) * 128], wt[:],
                                    start=(ib == 0), stop=(ib == IB - 1))
                        for tb in tbs:
                            so = st.tile([128, 512], bf16, name="stv")
                            nc.vector.tensor_copy(so[:], psums[tb][:])
                            nc.sync.dma_start(
                                VS[tb * 128:(tb + 1) * 128,
                                   og * 512:(og + 1) * 512], so[:])

            # ---------------- P2: attention ----------------
            if 2 in phases:
              with tc.tile_pool(name="qk", bufs=2) as qk, \
                 tc.tile_pool(name="rt", bufs=1) as rt, \
                 tc.tile_pool(name="tp", bufs=3) as tp, \
                 tc.tile_pool(name="ptp", bufs=2) as ptp, \
                 tc.tile_pool(name="ex", bufs=2) as exp_pool, \
                 tc.tile_pool(name="ob", bufs=2) as obp, \
                 tc.tile_pool(name="ps_s", bufs=4, space="PSUM") as ps_s, \
                 tc.tile_pool(name="ps_d", bufs=2, space="PSUM") as ps_d, \
                 tc.tile_pool(name="ps_o", bufs=2, space="PSUM") as ps_o:
                def rope_load(SRC, c0, roped, which, pos0=None):
                    """Load [128, HEADS, WIN] token window at c0 from SRC
                    (head-major scratch), apply RoPE into `roped` (f32r).
                    pos0: column into COS/SINS (halo coords); default c0.
                    cos/sin slices are DMA-replicated x4 so the DVE ops run
                    on [128, 4*WIN] four-head groups."""
                    if pos0 is None:
                        pos0 = c0
                    raw = rt.tile([128, HEADS, WIN], bf16, name=f"raw{which}")
                    rot = rt.tile([128, HEADS, WIN], bf16, name=f"rot{which}")
                    sl = SRC[:, :, c0:c0 + WIN]
                    nc.sync.dma_start(raw[:], sl.rearrange("h d w -> d h w"))
                    nc.sync.dma_start(rot[0:64], sl[:, 64:128, :].rearrange("h d w -> d h w"))
                    nc.sync.dma_start(rot[64:128], sl[:, 0:64, :].rearrange("h d w -> d h w"))
                    cos4 = tp.tile([128, 4, WIN], f32, name="cos4")
                    sin4 = tp.tile([128, 4, WIN], f32, name="sin4")
                    for g in range(4):
                        nc.sync.dma_start(cos4[:, g], COS[:, pos0:pos0 + WIN])
                        nc.sync.dma_start(sin4[:, g], SINS[:, pos0:pos0 + WIN])
                    for g in range(4):
                        gs = slice(g * 4, (g + 1) * 4)
                        tmp = tp.tile([128, 4, WIN], bf16, name="tmp")
                        nc.vector.tensor_mul(tmp[:], rot[:, gs], sin4[:])
                        nc.vector.tensor_mul(roped[:, gs], raw[:, gs], cos4[:])
                        nc.vector.tensor_add(roped[:, gs], roped[:, gs], tmp[:])

                kt_prev = qk.tile([128, HEADS, WIN], bf16, name="kt")
                rope_load(KTS, 0, kt_prev, "k")
                v_prev = qk.tile([128, 2, DIMS], bf16, name="v")
                nc.sync.dma_start(
                    v_prev[:], VS[0:WIN].rearrange("(tb p) c -> p tb c", p=128))

                for c in range(NC_):
                    kt_cur = qk.tile([128, HEADS, WIN], bf16, name="kt")
                    rope_load(KTS, WIN + c * WIN, kt_cur, "k")
                    v_cur = qk.tile([128, 2, DIMS], bf16, name="v")
                    nc.sync.dma_start(
                        v_cur[:], VS[WIN + c * WIN:WIN + (c + 1) * WIN]
                        .rearrange("(tb p) c -> p tb c", p=128))
                    qt = qk.tile([128, HEADS, WIN], bf16, name="qt")
                    rope_load(QTS, c * WIN, qt, "q", pos0=WIN + c * WIN)

                    kts = [kt_prev, kt_prev, kt_cur, kt_cur]
                    vs = [v_prev, v_prev, v_cur, v_cur]
                    W2 = 2 * WIN
                    for h0 in range(0, HEADS, 2):
                        # per head-pair psums: denominator and O^T share
                        # [128, 512] banks (head h0 left, h0+1 right)
                        pd = ps_d.tile([128, W2], f32, name="pd")
                        po = ps_o.tile([128, W2], f32, name="po")
                        pts2 = []
                        for h in (h0, h0 + 1):
                            # scores: kb0|kb1 pair and kb2|kb3 pair in one bank
                            pts = []
                            for pr in range(2):
                                ps = ps_s.tile([128, W2], f32, name="ps")
                                for kb2 in range(2):
                                    kb = pr * 2 + kb2
                                    nc.tensor.matmul(
                                        ps[:, kb2 * WIN:(kb2 + 1) * WIN],
                                        kts[kb][:, h, (kb % 2) * 128:(kb % 2) * 128 + 128],
                                        qt[:, h], start=True, stop=True)
                                pb = ptp.tile([128, W2], bf16, name=f"pt{pr}")
                                if pr == 0:
                                    if c == 0:
                                        nc.scalar.activation(pb[:], ps[:], AF.Exp,
                                                             bias=pgate[:], scale=ISQ)
                                    else:
                                        nc.scalar.activation(pb[:], ps[:], AF.Exp,
                                                             scale=ISQ)
                                else:
                                    ex = exp_pool.tile([128, W2], bf16, name="ex")
                                    nc.scalar.activation(ex[:], ps[:], AF.Exp,
                                                         scale=ISQ)
                                    nc.vector.tensor_mul(pb[:], ex[:], tri23[:])
                                pts.append(pb)
                            pts2.append(pts)

                        for i, h in enumerate((h0, h0 + 1)):
                            sl = slice(i * WIN, (i + 1) * WIN)
                            for kb in range(4):
                                pb = pts2[i][kb // 2][:, (kb % 2) * WIN:(kb % 2 + 1) * WIN]
                                nc.tensor.matmul(pd[:, sl], onesm[:], pb,
                                                 start=(kb == 0), stop=(kb == 3))
                            for kb in range(4):
                                pb = pts2[i][kb // 2][:, (kb % 2) * WIN:(kb % 2 + 1) * WIN]
                                nc.tensor.matmul(
                                    po[:, sl], vs[kb][:, kb % 2, h * 128:(h + 1) * 128],
                                    pb, start=(kb == 0), stop=(kb == 3))
                        rb = obp.tile([128, W2], f32, name="rb")
                        with nc.allow_low_precision("softmax denominator"):
                            nc.vector.reciprocal(rb[:], pd[:])
                        ot = obp.tile([128, W2], f32r, name="ot")
                        nc.vector.tensor_mul(ot[:], po[:], rb[:])
                        nc.sync.dma_start(OTS[h0][:, c * WIN:(c + 1) * WIN],
                                          ot[:, 0:WIN])
                        nc.sync.dma_start(OTS[h0 + 1][:, c * WIN:(c + 1) * WIN],
                                          ot[:, WIN:W2])
                    kt_prev, v_prev = kt_cur, v_cur

            # ---------------- P3: output projection ----------------
            if 3 in phases:
              with tc.tile_pool(name="p3", bufs=1) as p3, \
                 tc.tile_pool(name="otp", bufs=3) as otp, \
                 tc.tile_pool(name="so3", bufs=6) as so3, \
                 tc.tile_pool(name="pp3", bufs=8, space="PSUM") as pp3:
                wo = p3.tile([128, IB, DIMS], f32r)
                nc.sync.dma_start(wo[:], WO.rearrange("(ib p) d -> p ib d", p=128))
                for tt in range(T // 128):
                    ots = otp.tile([128, HEADS, 128], f32r, name="ots")
                    nc.sync.dma_start(
                        ots[:], OTS[:, :, tt * 128:(tt + 1) * 128]
                        .rearrange("h d w -> d h w"))
                    for nt in range(4):
                        ps = pp3.tile([128, 512], f32, name="pp3")
                        for h in range(HEADS):
                            nc.tensor.matmul(
                                ps[:], ots[:, h], wo[:, h, nt * 512:(nt + 1) * 512],
                                start=(h == 0), stop=(h == HEADS - 1))
                        so = so3.tile([128, 512], f32, name="so")
                        nc.scalar.copy(so[:], ps[:])
                        nc.sync.dma_start(
                            OUT[tt * 128:(tt + 1) * 128,
                                nt * 512:(nt + 1) * 512], so[:])
    return nc


def _host_inputs(hidden_states, Wq, Wk, Wv, Wo, T):
    """Build the 8 per-core input maps."""
    TH = T + WIN
    inv_freq = 1.0 / (THETA ** (np.arange(0, HD, 2, dtype=np.float32) / HD))

    qq = np.arange(WIN)[None, :]
    kk = np.arange(128)[:, None]
    tri23 = np.concatenate([(qq >= kk), (qq >= kk + 128)], 1).astype(ml_dtypes.bfloat16)
    onesm_bf = np.ones((128, 128), ml_dtypes.bfloat16)

    Wq, Wk, Wv, Wo = (np.ascontiguousarray(w, np.float32) for w in (Wq, Wk, Wv, Wo))
    in_maps = []
    for core in range(8):
        b, sh = divmod(core, NSH)
        t0 = sh * T
        hs = np.zeros((TH, DIMS), np.float32)
        lo = max(0, t0 - WIN)
        hs[WIN - (t0 - lo):] = hidden_states[b, lo:t0 + T]
        hT = np.ascontiguousarray(hs.T)

        pos = np.arange(t0 - WIN, t0 + T, dtype=np.float32)
        f = np.outer(inv_freq, pos)                      # [64, TH]
        cos = np.concatenate([np.cos(f), np.cos(f)], 0)  # [128, TH]
        sin = np.sin(f)
        sins = np.concatenate([-sin, sin], 0)
        pg = np.full((128, 1), -1e30 if sh == 0 else 0.0, np.float32)
        in_maps.append({
            "HT": hT, "WQ": Wq, "WK": Wk, "WV": Wv, "WO": Wo,
            "COS": cos.astype(np.float32), "SINS": sins.astype(np.float32),
            "TRI23": tri23, "PGATE": pg, "ONESM": onesm_bf,
        })
    return in_maps


_CACHE = {}


def run(hidden_states, Wq, Wk, Wv, Wo, T=S // NSH, **spmd_kwargs):
    key = T
    if key not in _CACHE:
        nc = bacc.Bacc(None)
        build(nc, T)
        nc.finalize()
        _CACHE[key] = nc
    nc = _CACHE[key]
    in_maps = _host_inputs(hidden_states, Wq, Wk, Wv, Wo, T)
    res = run_bass_kernel_spmd(nc, in_maps, core_ids=list(range(8)), **spmd_kwargs)
    outs = [res.results[i]["OUT"] for i in range(8)]
    full = np.empty((B, NSH * T, DIMS), np.float32)
    for core in range(8):
        b, sh = divmod(core, NSH)
        full[b, sh * T:(sh + 1) * T] = outs[core]
    return full, res


def kernel(hidden_states, Wq, Wk, Wv, Wo):
    out, _ = run(np.asarray(hidden_states), Wq, Wk, Wv, Wo)
    return out
